# revision 1
# baseline (speedup 1.0000x reference)
"""CharEmb kernel for Trainium2 (8 NeuronCores, batch-sharded).

Computation (per word of 32 chars):
  emb = table[ids]                  # [32 chars, 64] gathered fp32
  x[i, j] = emb[i//2, 32*(i%2)+j]   # raw-buffer reshape [64, 32]
  y[f, t] = sum_{i,k} x[i, t+k] * w[f, i, k]   (valid conv, K=3)
  out[f] = max_t y[f, t] + b[f]

Device mapping per core (2048 words = 65536 chars):
  - dma_gather: char c -> partition c%128, its 64-fp32 table row on the
    free dim.  A 128-partition block = 4 words (slot s = partitions
    32s..32s+32 = word 4b+s of block b).
  - conv: contraction over (p, h, k) = (char-in-word, half, tap) as 6
    accumulating K=32 matmuls per word-slot, row-tiled across the 4
    slots via tile_position.  rhs column j0=32h+k..+30 of the gathered
    rows; stationary W[h,k][p, f] = conv_w[f, 2p+h, k].
  - maxpool: per-word tensor_reduce(max) over the 30 t columns in PSUM.
"""

import sys
from contextlib import ExitStack

import numpy as np

if "/opt/trn_rl_repo" not in sys.path:
    sys.path.insert(0, "/opt/trn_rl_repo")

import concourse.bass as bass
import concourse.tile as tile
from concourse import bacc, mybir
from concourse.bass_utils import run_bass_kernel_spmd

# Problem constants (hardcoded per spec)
B, S, C = 32, 512, 32
V, E = 101, 64
F, K = 128, 3
T = C - K + 1  # 30 valid conv positions
NCORES = 8
WORDS = (B * S) // NCORES  # 2048 words per core
NCHARS = WORDS * C  # 65536

CHUNK_WORDS = 64  # words per pipeline chunk
NCHUNKS = WORDS // CHUNK_WORDS  # 32
BLOCKS = CHUNK_WORDS // 4  # 16 gather blocks (128 chars) per chunk
CHUNK_IDX_COLS = (CHUNK_WORDS * C) // 16  # 128 idx columns per chunk

f32 = mybir.dt.float32
f32r = mybir.dt.float32r
bf16 = mybir.dt.bfloat16
i16 = mybir.dt.int16


def build_kernel(words=WORDS, chunk_words=CHUNK_WORDS, num_devices=NCORES,
                 debug_obuf=False, add_bias=True):
    nchunks = words // chunk_words
    blocks = chunk_words // 4
    idx_cols_per_chunk = (chunk_words * C) // 16

    nc = bacc.Bacc(
        "TRN2",
        target_bir_lowering=False,
        debug=False,
        enable_asserts=True,
        num_devices=num_devices,
    )

    idx_d = nc.dram_tensor("idx", [128, (words * C) // 16], i16, kind="ExternalInput")
    tab_d = nc.dram_tensor("tab", [V, E], f32r, kind="ExternalInput")
    w_d = nc.dram_tensor("wmat", [128, 6 * 128], f32r, kind="ExternalInput")
    b_d = nc.dram_tensor("bias", [128, 1], f32, kind="ExternalInput")
    # f-major output: out[f, col] with col = 64c + 16s + w -> word 64c + 4w + s
    out_d = nc.dram_tensor("out", [128, words], f32, kind="ExternalOutput")
    if debug_obuf:
        dbg_d = nc.dram_tensor("dbg_obuf", [128, words], f32, kind="ExternalOutput")

    with tile.TileContext(nc) as tc, ExitStack() as ctx:
        const_pool = ctx.enter_context(tc.tile_pool(name="const", bufs=1))
        g_pool = ctx.enter_context(tc.tile_pool(name="gath", bufs=3))
        p_pool = ctx.enter_context(tc.tile_pool(name="psum", bufs=2, space="PSUM"))

        idx_sb = const_pool.tile([128, (words * C) // 16], i16)
        w_sb = const_pool.tile([128, 6 * 128], f32r)
        b_sb = const_pool.tile([128, 1], f32)
        obuf = const_pool.tile([128, words], f32)

        nc.sync.dma_start(idx_sb[:], idx_d.ap())
        nc.sync.dma_start(w_sb[:], w_d.ap())
        nc.sync.dma_start(b_sb[:], b_d.ap())

        for c in range(nchunks):
            # --- gather embeddings for this chunk's 2048 chars ---
            g = g_pool.tile([128, blocks * E], f32r)
            g_r = g[:].rearrange("p (b e) -> p b e", e=E)
            nc.gpsimd.dma_gather(
                out_ap=g_r,
                in_ap=tab_d.ap(),
                idxs_ap=idx_sb[:, c * idx_cols_per_chunk:(c + 1) * idx_cols_per_chunk],
                num_idxs=chunk_words * C,
                num_idxs_reg=chunk_words * C,
                elem_size=E,
                single_packet=False,
            )

            # --- conv: 6 accumulating matmuls x 4 row-tiled slots ---
            p = p_pool.tile([128, 4 * 512], f32)
            for hk in range(6):
                h, k = divmod(hk, 3)
                j0 = 32 * h + k
                for s in range(4):
                    out_ap = (
                        p[:, 512 * s:512 * s + blocks * T]
                        .rearrange("f (w t) -> f w t", t=T)
                    )
                    rhs = g_r[32 * s:32 * s + 32, :, j0:j0 + T]
                    lhsT = w_sb[32 * s:32 * s + 32, 128 * hk:128 * hk + 128]
                    nc.tensor.matmul(
                        out_ap,
                        lhsT,
                        rhs,
                        start=(hk == 0),
                        stop=(hk == 5),
                        tile_position=(32 * s, 0),
                        skip_group_check=True,
                    )

            # --- maxpool over t (per word) ---
            p_v = (
                p[:].rearrange("f (s x) -> f s x", x=512)[:, :, 0:blocks * T]
                .rearrange("f s (w t) -> f s w t", t=T)
            )
            o_v = (
                obuf[:, c * chunk_words:(c + 1) * chunk_words]
                .rearrange("f (s w) -> f s w", w=blocks)
            )
            nc.vector.tensor_reduce(
                o_v, p_v, axis=mybir.AxisListType.X, op=mybir.AluOpType.max
            )

        # --- bias + store ---
        if debug_obuf:
            nc.sync.dma_start(dbg_d.ap(), obuf[:])
        if add_bias:
            nc.vector.tensor_scalar_add(obuf[:], obuf[:], b_sb[:, 0:1])
        nc.sync.dma_start(out_d.ap(), obuf[:])

    nc.compile()
    return nc


def host_prep(char_ids, emb_table, conv_w, conv_b, words=WORDS, num_devices=NCORES):
    """Build per-core input maps from full inputs."""
    char_ids = np.asarray(char_ids)
    emb_table = np.ascontiguousarray(np.asarray(emb_table), dtype=np.float32)
    conv_w = np.asarray(conv_w, dtype=np.float32)
    conv_b = np.asarray(conv_b, dtype=np.float32)

    ids_flat = char_ids.reshape(-1, C).astype(np.int16)  # [16384, 32]

    # stationary weights: wmat[32s+p, 128*(3h+k) + f] = conv_w[f, 2p+h, k]
    wmat = np.zeros((128, 6 * 128), dtype=np.float32)
    for h in range(2):
        for k in range(3):
            hk = 3 * h + k
            w_pf = conv_w[:, h::2, k].T  # [32 p, 128 f]
            wmat[:, 128 * hk:128 * (hk + 1)] = np.tile(w_pf, (4, 1))

    bias = conv_b.reshape(128, 1)

    in_maps = []
    for j in range(num_devices):
        ids_core = ids_flat[j * words:(j + 1) * words]  # [words, 32]
        flat = ids_core.reshape(-1)  # char-major
        # wrap: char i -> [i%16, i//16], chunk-local columns
        ncols = flat.size // 16
        wrapped = flat.reshape(ncols, 16).T.copy()  # [16, ncols]
        idx = np.tile(wrapped, (8, 1))  # replicate to 128 partitions
        in_maps.append(
            {
                "idx": np.ascontiguousarray(idx),
                "tab": emb_table,
                "wmat": wmat,
                "bias": bias,
            }
        )
    return in_maps


def _ensure_ntff_hook():
    """The agent image's antenv lacks axon_hooks; shim it and install the
    ctypes NTFF profiling hook so trace=True yields HW exec times."""
    import types

    if "antenv.axon_hooks" in sys.modules:
        return
    mod = types.ModuleType("antenv.axon_hooks")
    _hook = [None]
    mod.get_axon_ntff_profile_hook = lambda: _hook[0]
    mod.set_axon_ntff_profile_hook = lambda h: _hook.__setitem__(0, h)
    sys.modules["antenv.axon_hooks"] = mod
    try:
        import antenv

        antenv.axon_hooks = mod
        from trn_agent_boot.trn_boot import _ntff_profile_via_ctypes

        hook = _ntff_profile_via_ctypes("/opt/axon/libaxon_pjrt.so")
        mod.set_axon_ntff_profile_hook(hook)
    except Exception as e:  # degrade to no-trace
        print(f"ntff hook install failed: {e}", file=sys.stderr)


_NC_CACHE = {}


def _get_nc():
    if "nc" not in _NC_CACHE:
        _NC_CACHE["nc"] = build_kernel()
    return _NC_CACHE["nc"]


def unscramble_out(raw, words=WORDS, chunk_words=CHUNK_WORDS):
    """[128 f, words] f-major, col = 64c+16s+w  ->  [words, 128] word-major."""
    blocks = chunk_words // 4
    nchunks = words // chunk_words
    o = raw.reshape(128, nchunks, 4, blocks)  # [f, c, s, w]
    o = o.transpose(1, 3, 2, 0)  # [c, w, s, f]; word = 64c + 4w + s
    return np.ascontiguousarray(o.reshape(words, 128))


def kernel(char_ids, emb_table, conv_w, conv_b, trace=False):
    if trace:
        _ensure_ntff_hook()
    nc = _get_nc()
    in_maps = host_prep(char_ids, emb_table, conv_w, conv_b)
    res = run_bass_kernel_spmd(
        nc, in_maps, core_ids=list(range(NCORES)), trace=trace
    )
    outs = [unscramble_out(res.results[j]["out"]) for j in range(NCORES)]
    full = np.concatenate(outs, axis=0).reshape(B, S, F).astype(np.float32)
    if trace:
        return full, res
    return full



# revision 3
# speedup vs baseline: 1.8751x; 1.8751x over previous
"""CharEmb kernel for Trainium2 (8 NeuronCores, batch-sharded).

Computation (per word of 32 chars):
  emb = table[ids]                  # [32 chars, 64] gathered
  x[i, j] = emb[i//2, 32*(i%2)+j]   # raw-buffer reshape [64, 32]
  y[f, t] = sum_{i,k} x[i, t+k] * w[f, i, k]   (valid conv, K=3)
  out[f] = max_t y[f, t] + b[f]

v2: pair-table gather + bf16 matmuls.
  - The embedding gather is Q7 descriptor-gen bound (~8 ns/idx on the
    GpSimd SWDGE path), so we halve the index count: a PAIR table
    ptab[v1*101+v2] = [table[v1] ; table[v2]] (128 bf16 = 256 B rows).
    One gather element covers char c of two adjacent words (2P, 2P+1).
  - Element (blk, j, c) -> partition 32j+c, block blk.  A 128-partition
    block = 4 pairs = 8 words.  Free dim: 128*blk + 64*o + u for word
    parity o, emb dim u.  With w-hat = 2*blk+o the rhs AP matches the
    single-char layout exactly (stride 64).
  - conv: 6 accumulating bf16 K=32 matmuls per word-slot, row-tiled
    across the 4 slots via tile_position; rhs cols j0=32h+k..+30.
  - maxpool: per-word tensor_reduce(max) over the 30 t cols in PSUM.
"""

import sys
from contextlib import ExitStack

import numpy as np

if "/opt/trn_rl_repo" not in sys.path:
    sys.path.insert(0, "/opt/trn_rl_repo")

import concourse.bass as bass
import concourse.tile as tile
from concourse import bacc, mybir
from concourse.bass_utils import run_bass_kernel_spmd

# Problem constants (hardcoded per spec)
B, S, C = 32, 512, 32
V, E = 101, 64
F, K = 128, 3
T = C - K + 1  # 30 valid conv positions
NCORES = 8
WORDS = (B * S) // NCORES  # 2048 words per core
NPAIRS = WORDS // 2  # 1024 word-pairs per core
NIDX = NPAIRS * C  # 32768 gather elements per core

GATHER_WORDS = 128  # words per gather instruction (2048 idxs)
CHUNK_WORDS = 64  # words per conv/psum chunk
NCHUNKS = WORDS // CHUNK_WORDS  # 32
NGATHERS = WORDS // GATHER_WORDS  # 16
BLOCKS = CHUNK_WORDS // 8  # 8 gather blocks (128 pair-elems) per conv chunk
CHUNK_IDX = (CHUNK_WORDS // 2) * C  # 1024 idxs per conv chunk

f32 = mybir.dt.float32
bf16 = mybir.dt.bfloat16
i16 = mybir.dt.int16


def build_kernel(words=WORDS, num_devices=NCORES, add_bias=True):
    nchunks = words // CHUNK_WORDS
    ngathers = words // GATHER_WORDS
    chunks_per_gather = GATHER_WORDS // CHUNK_WORDS  # 2

    nc = bacc.Bacc(
        "TRN2",
        target_bir_lowering=False,
        debug=False,
        enable_asserts=True,
        num_devices=num_devices,
    )

    idx_d = nc.dram_tensor("idx", [128, (words // 2 * C) // 16], i16, kind="ExternalInput")
    tab_d = nc.dram_tensor("tab", [V * V, 2 * E], bf16, kind="ExternalInput")
    w_d = nc.dram_tensor("wmat", [128, 6 * 128], bf16, kind="ExternalInput")
    b_d = nc.dram_tensor("bias", [128, 1], f32, kind="ExternalInput")
    # f-major output: out[f, col] with col = 64c + 16j + what
    #   -> word 64c + 8*(what//2) + 2j + what%2
    out_d = nc.dram_tensor("out", [128, words], f32, kind="ExternalOutput")

    with tile.TileContext(nc) as tc, ExitStack() as ctx:
        const_pool = ctx.enter_context(tc.tile_pool(name="const", bufs=1))
        g_pool = ctx.enter_context(tc.tile_pool(name="gath", bufs=3))
        p_pool = ctx.enter_context(tc.tile_pool(name="psum", bufs=2, space="PSUM"))

        idx_sb = const_pool.tile([128, (words // 2 * C) // 16], i16)
        w_sb = const_pool.tile([128, 6 * 128], bf16)
        b_sb = const_pool.tile([128, 1], f32)
        obuf = const_pool.tile([128, words], f32)

        nc.sync.dma_start(idx_sb[:], idx_d.ap())
        nc.sync.dma_start(w_sb[:], w_d.ap())
        nc.sync.dma_start(b_sb[:], b_d.ap())

        gcols = GATHER_WORDS // 2 * C // 16  # idx cols per gather (128)
        for gi in range(ngathers):
            # --- gather pair-embeddings for 128 words (2048 idxs) ---
            g = g_pool.tile([128, 16 * 2 * E], bf16)  # 16 blocks x 256B
            g_r = g[:].rearrange("p (b e) -> p b e", e=2 * E)
            nc.gpsimd.dma_gather(
                out_ap=g_r,
                in_ap=tab_d.ap(),
                idxs_ap=idx_sb[:, gi * gcols:(gi + 1) * gcols],
                num_idxs=GATHER_WORDS // 2 * C,
                num_idxs_reg=GATHER_WORDS // 2 * C,
                elem_size=2 * E,
                single_packet=False,
            )
            # view with 64-wide granularity: [128, 32 what, 64]
            g_w = g[:].rearrange("p (x j) -> p x j", j=E)

            for ci in range(chunks_per_gather):
                c = gi * chunks_per_gather + ci
                # --- conv: 6 accumulating matmuls x 4 row-tiled slots ---
                p = p_pool.tile([128, 4 * 512], f32)
                for hk in range(6):
                    h, k = divmod(hk, 3)
                    j0 = 32 * h + k
                    for s in range(4):
                        out_ap = (
                            p[:, 512 * s:512 * s + 16 * T]
                            .rearrange("f (w t) -> f w t", t=T)
                        )
                        rhs = g_w[32 * s:32 * s + 32,
                                  ci * 16:(ci + 1) * 16, j0:j0 + T]
                        lhsT = w_sb[32 * s:32 * s + 32, 128 * hk:128 * hk + 128]
                        nc.tensor.matmul(
                            out_ap,
                            lhsT,
                            rhs,
                            start=(hk == 0),
                            stop=(hk == 5),
                            tile_position=(32 * s, 0),
                            skip_group_check=True,
                        )

                # --- maxpool over t (per word) ---
                p_v = (
                    p[:].rearrange("f (s x) -> f s x", x=512)[:, :, 0:16 * T]
                    .rearrange("f s (w t) -> f s w t", t=T)
                )
                o_v = (
                    obuf[:, c * CHUNK_WORDS:(c + 1) * CHUNK_WORDS]
                    .rearrange("f (s w) -> f s w", w=16)
                )
                nc.vector.tensor_reduce(
                    o_v, p_v, axis=mybir.AxisListType.X, op=mybir.AluOpType.max
                )

        # --- bias + store ---
        if add_bias:
            nc.vector.tensor_scalar_add(obuf[:], obuf[:], b_sb[:, 0:1])
        nc.sync.dma_start(out_d.ap(), obuf[:])

    nc.compile()
    return nc


def host_prep(char_ids, emb_table, conv_w, conv_b, words=WORDS, num_devices=NCORES):
    """Build per-core input maps from full inputs."""
    char_ids = np.asarray(char_ids)
    emb_table = np.asarray(emb_table, dtype=np.float32)
    conv_w = np.asarray(conv_w, dtype=np.float32)
    conv_b = np.asarray(conv_b, dtype=np.float32)

    # pair table: ptab[v1*101+v2] = [table[v1] ; table[v2]] in bf16
    from ml_dtypes import bfloat16 as np_bf16

    tab_bf = emb_table.astype(np_bf16)  # [101, 64]
    ptab = np.zeros((V * V, 2 * E), dtype=np_bf16)
    ptab_v = ptab.reshape(V, V, 2, E)
    ptab_v[:, :, 0, :] = tab_bf[:, None, :]
    ptab_v[:, :, 1, :] = tab_bf[None, :, :]
    ptab = np.ascontiguousarray(ptab_v.reshape(V * V, 2 * E))

    ids_flat = char_ids.reshape(-1, C).astype(np.int32)  # [16384, 32]

    # stationary weights: wmat[32s+p, 128*(3h+k) + f] = conv_w[f, 2p+h, k]
    wmat = np.zeros((128, 6 * 128), dtype=np.float32)
    for h in range(2):
        for k in range(3):
            hk = 3 * h + k
            w_pf = conv_w[:, h::2, k].T  # [32 p, 128 f]
            wmat[:, 128 * hk:128 * (hk + 1)] = np.tile(w_pf, (4, 1))
    wmat = wmat.astype(np_bf16)

    bias = conv_b.reshape(128, 1)

    in_maps = []
    for j in range(num_devices):
        ids_core = ids_flat[j * words:(j + 1) * words]  # [words, 32]
        # pair index stream: pid[P, c] = ids[2P, c]*101 + ids[2P+1, c]
        pid = ids_core[0::2] * V + ids_core[1::2]  # [1024 pairs, 32]
        # element order m = 128*blk + 32*jh + c with pair P = 4*blk + jh
        # (per 128-word gather group: blk in [0,16), jh in [0,4))
        pid_g = pid.reshape(-1, 16, 4, C)  # [groups, blk, jh, c]
        flat = pid_g.transpose(0, 1, 2, 3).reshape(-1)  # m-order: blk, jh, c
        flat = flat.astype(np.int16)
        ncols = flat.size // 16
        wrapped = flat.reshape(ncols, 16).T.copy()  # [16, ncols]
        idx = np.tile(wrapped, (8, 1))  # replicate to 128 partitions
        in_maps.append(
            {
                "idx": np.ascontiguousarray(idx),
                "tab": ptab,
                "wmat": wmat,
                "bias": bias,
            }
        )
    return in_maps


def _ensure_ntff_hook():
    """The agent image's antenv lacks axon_hooks; shim it and install the
    ctypes NTFF profiling hook so trace=True yields HW exec times."""
    import types

    if "antenv.axon_hooks" in sys.modules:
        return
    mod = types.ModuleType("antenv.axon_hooks")
    _hook = [None]
    mod.get_axon_ntff_profile_hook = lambda: _hook[0]
    mod.set_axon_ntff_profile_hook = lambda h: _hook.__setitem__(0, h)
    sys.modules["antenv.axon_hooks"] = mod
    try:
        import antenv

        antenv.axon_hooks = mod
        from trn_agent_boot.trn_boot import _ntff_profile_via_ctypes

        hook = _ntff_profile_via_ctypes("/opt/axon/libaxon_pjrt.so")
        mod.set_axon_ntff_profile_hook(hook)
    except Exception as e:  # degrade to no-trace
        print(f"ntff hook install failed: {e}", file=sys.stderr)


_NC_CACHE = {}


def _get_nc():
    if "nc" not in _NC_CACHE:
        _NC_CACHE["nc"] = build_kernel()
    return _NC_CACHE["nc"]


def unscramble_out(raw, words=WORDS):
    """[128 f, words], col = 64c + 16j + what, what = 2*blk + o
    -> word 64c + 8*blk + 2*j + o."""
    nchunks = words // CHUNK_WORDS
    o = raw.reshape(128, nchunks, 4, 8, 2)  # [f, c, j, blk, o]
    o = o.transpose(1, 3, 2, 4, 0)  # [c, blk, j, o, f]
    return np.ascontiguousarray(o.reshape(words, 128))


def kernel(char_ids, emb_table, conv_w, conv_b, trace=False):
    if trace:
        _ensure_ntff_hook()
    nc = _get_nc()
    in_maps = host_prep(char_ids, emb_table, conv_w, conv_b)
    res = run_bass_kernel_spmd(
        nc, in_maps, core_ids=list(range(NCORES)), trace=trace
    )
    outs = [unscramble_out(res.results[j]["out"]) for j in range(NCORES)]
    full = np.concatenate(outs, axis=0).reshape(B, S, F).astype(np.float32)
    if trace:
        return full, res
    return full


# revision 12
# speedup vs baseline: 2.0526x; 1.0947x over previous
"""CharEmb kernel for Trainium2 (8 NeuronCores, batch-sharded).

Computation (per word of 32 chars):
  emb = table[ids]                  # [32 chars, 64] gathered
  x[i, j] = emb[i//2, 32*(i%2)+j]   # raw-buffer reshape [64, 32]
  y[f, t] = sum_{i,k} x[i, t+k] * w[f, i, k]   (valid conv, K=3)
  out[f] = max_t y[f, t] + b[f]

v3: hybrid pair-table SWDGE gather + PE one-hot lookup.
  - The SWDGE gather path is Q7 descriptor-gen bound (~8 ns/idx), so the
    embedding lookup is split across two engines:
      * GpSimd: pair-table gather.  ptab[v1*101+v2] = [table[v1];table[v2]]
        (256-B bf16 rows); one element covers char c of words (2P, 2P+1).
        Handles blocks 0..BS-1 of each 16-block (128-word) group.
      * TensorE: one-hot lookup for blocks BS..15.  Per what-column
        (4 words), stationary = host-built one-hot [101, 128] and moving =
        the bf16 table [101, 64]: out[32j+c, u] = table[ids[w(j),c], u] in
        PSUM, copied to the gather tile by the (idle) Scalar engine.
  - Layout: element/col (blk, j, c) -> partition 32j+c.  A 128-partition
    block = 8 words; free dim 128*blk + 64*o + u, what = 2*blk+o.
  - conv: 6 accumulating bf16 K=32 matmuls per word-slot, row-tiled via
    tile_position, one single-bank PSUM tile per slot (pool of 6).
  - maxpool: per-(chunk, slot) tensor_reduce(max) over the 30 t cols.
"""

import sys
from contextlib import ExitStack

import numpy as np

if "/opt/trn_rl_repo" not in sys.path:
    sys.path.insert(0, "/opt/trn_rl_repo")

import concourse.bass as bass
import concourse.tile as tile
from concourse import bacc, mybir
from concourse.bass_utils import run_bass_kernel_spmd

# Problem constants (hardcoded per spec)
B, S, C = 32, 512, 32
V, E = 101, 64
F, K = 128, 3
T = C - K + 1  # 30 valid conv positions
NCORES = 8
WORDS = (B * S) // NCORES  # 2048 words per core

GATHER_WORDS = 128  # words per gather group
NGROUPS = WORDS // GATHER_WORDS  # 16
CHUNK_WORDS = 64  # words per conv/psum chunk
NCHUNKS = WORDS // CHUNK_WORDS  # 32

BS = 10  # blocks (of 16) per group fetched via SWDGE; rest via PE one-hot
PE_BLOCKS = (16 - BS) * 2  # one-hot what-columns per group (12)
GIDX = BS * 128  # SWDGE idxs per group
GCOLS = GIDX // 16  # idx columns per group

f32 = mybir.dt.float32
bf16 = mybir.dt.bfloat16
i16 = mybir.dt.int16


def build_kernel(num_devices=NCORES, add_bias=True):
    words = WORDS

    nc = bacc.Bacc(
        "TRN2",
        target_bir_lowering=False,
        debug=False,
        enable_asserts=True,
        num_devices=num_devices,
    )

    idx_d = nc.dram_tensor("idx", [128, NGROUPS * GCOLS], i16, kind="ExternalInput")
    tab_d = nc.dram_tensor("tab", [V * V, 2 * E], bf16, kind="ExternalInput")
    tabs_d = nc.dram_tensor("tabs", [V, E], bf16, kind="ExternalInput")
    oh_d = nc.dram_tensor(
        "oh", [V, NGROUPS * PE_BLOCKS * 128], bf16, kind="ExternalInput"
    )
    w_d = nc.dram_tensor("wmat", [128, 6 * 128], bf16, kind="ExternalInput")
    b_d = nc.dram_tensor("bias", [128, 1], f32, kind="ExternalInput")
    # f-major output: out[f, col] with col = 64c + 16j + what
    #   -> word 64c + 8*(what//2) + 2j + what%2
    out_d = nc.dram_tensor("out", [128, words], f32, kind="ExternalOutput")

    with tile.TileContext(nc) as tc, ExitStack() as ctx:
        const_pool = ctx.enter_context(tc.tile_pool(name="const", bufs=1))
        g_pool = ctx.enter_context(tc.tile_pool(name="gath", bufs=3))
        oh_pool = ctx.enter_context(tc.tile_pool(name="oh", bufs=3))
        pA_pool = ctx.enter_context(tc.tile_pool(name="psA", bufs=6, space="PSUM"))
        pB_pool = ctx.enter_context(tc.tile_pool(name="psB", bufs=2, space="PSUM"))

        idx_sb = const_pool.tile([128, NGROUPS * GCOLS], i16)
        tabs_sb = const_pool.tile([128, E], bf16)
        w_sb = const_pool.tile([128, 6 * 128], bf16)
        b_sb = const_pool.tile([128, 1], f32)
        obuf = const_pool.tile([128, words], f32)

        # per-group idx slices so the first gather starts early
        for gi in range(NGROUPS):
            nc.sync.dma_start(
                idx_sb[:, gi * GCOLS:(gi + 1) * GCOLS],
                idx_d.ap()[:, gi * GCOLS:(gi + 1) * GCOLS],
            )
        nc.sync.dma_start(tabs_sb[0:V, :], tabs_d.ap())
        nc.sync.dma_start(w_sb[:], w_d.ap())
        nc.sync.dma_start(b_sb[:], b_d.ap())

        ohcols = PE_BLOCKS * 128  # one-hot cols per group

        for gi in range(NGROUPS):
            # --- SWDGE gather: pair-embeddings for blocks 0..BS-1 ---
            g = g_pool.tile([128, 16 * 2 * E], bf16)  # 16 blocks x 256B
            g_r = g[:, 0:BS * 2 * E].rearrange("p (b e) -> p b e", e=2 * E)
            nc.gpsimd.dma_gather(
                out_ap=g_r,
                in_ap=tab_d.ap(),
                idxs_ap=idx_sb[:, gi * GCOLS:(gi + 1) * GCOLS],
                num_idxs=GIDX,
                num_idxs_reg=GIDX,
                elem_size=2 * E,
                single_packet=False,
            )
            # unified view, 64-wide what-columns: [128, 32 what, 64]
            g_w = g[:].rearrange("p (x j) -> p x j", j=E)

            # --- PE one-hot lookup for blocks BS..15 (PE_BLOCKS cols) ---
            oh_sb = oh_pool.tile([128, ohcols], bf16)
            nc.sync.dma_start(
                oh_sb[0:V, :], oh_d.ap()[:, gi * ohcols:(gi + 1) * ohcols]
            )
            n_banks = (PE_BLOCKS + 7) // 8
            pbs = [
                pB_pool.tile([128, 512], f32, name="pb")
                for _ in range(n_banks)
            ]
            for b in range(PE_BLOCKS):
                pb = pbs[b // 8]
                nc.tensor.matmul(
                    pb[:, (b % 8) * E:(b % 8) * E + E],
                    oh_sb[0:V, b * 128:(b + 1) * 128],
                    tabs_sb[0:V, :],
                    start=True,
                    stop=True,
                )
            for b in range(PE_BLOCKS):
                pb = pbs[b // 8]
                x = 2 * BS + b
                nc.scalar.copy(
                    g[:, E * x:E * x + E], pb[:, (b % 8) * E:(b % 8) * E + E]
                )

            # --- conv + maxpool, two 64-word chunks per group ---
            for ci in range(2):
                c = gi * 2 + ci
                pts = [
                    pA_pool.tile([128, 512], f32, name="pa")
                    for _ in range(4)
                ]
                for hk in range(6):
                    h, k = divmod(hk, 3)
                    j0 = 32 * h + k
                    for s in range(4):
                        out_ap = (
                            pts[s][:, 0:16 * T]
                            .rearrange("f (w t) -> f w t", t=T)
                        )
                        rhs = g_w[32 * s:32 * s + 32,
                                  ci * 16:(ci + 1) * 16, j0:j0 + T]
                        lhsT = w_sb[32 * s:32 * s + 32, 128 * hk:128 * hk + 128]
                        nc.tensor.matmul(
                            out_ap,
                            lhsT,
                            rhs,
                            start=(hk == 0),
                            stop=(hk == 5),
                            tile_position=(32 * s, 0),
                            skip_group_check=True,
                        )

                for s in range(4):
                    p_v = (
                        pts[s][:, 0:16 * T]
                        .rearrange("f (w t) -> f w t", t=T)
                    )
                    o_v = obuf[:, c * CHUNK_WORDS + 16 * s:
                               c * CHUNK_WORDS + 16 * s + 16]
                    nc.vector.tensor_reduce(
                        o_v, p_v, axis=mybir.AxisListType.X,
                        op=mybir.AluOpType.max,
                    )

                # --- bias + store per 8 chunks (512 words), overlapped ---
                if (c + 1) % 8 == 0:
                    q0 = (c + 1 - 8) * CHUNK_WORDS
                    q1 = (c + 1) * CHUNK_WORDS
                    if add_bias:
                        nc.vector.tensor_scalar_add(
                            obuf[:, q0:q1], obuf[:, q0:q1], b_sb[:, 0:1]
                        )
                    nc.sync.dma_start(out_d.ap()[:, q0:q1], obuf[:, q0:q1])

    nc.compile()
    return nc


def host_prep(char_ids, emb_table, conv_w, conv_b, num_devices=NCORES):
    """Build per-core input maps from full inputs."""
    from ml_dtypes import bfloat16 as np_bf16

    words = WORDS
    char_ids = np.asarray(char_ids)
    emb_table = np.asarray(emb_table, dtype=np.float32)
    conv_w = np.asarray(conv_w, dtype=np.float32)
    conv_b = np.asarray(conv_b, dtype=np.float32)

    tab_bf = emb_table.astype(np_bf16)  # [101, 64]
    # pair table: ptab[v1*101+v2] = [table[v1] ; table[v2]]
    ptab = np.zeros((V, V, 2, E), dtype=np_bf16)
    ptab[:, :, 0, :] = tab_bf[:, None, :]
    ptab[:, :, 1, :] = tab_bf[None, :, :]
    ptab = np.ascontiguousarray(ptab.reshape(V * V, 2 * E))

    ids_flat = char_ids.reshape(-1, C).astype(np.int32)  # [16384, 32]

    # stationary weights: wmat[32s+p, 128*(3h+k) + f] = conv_w[f, 2p+h, k]
    wmat = np.zeros((128, 6 * 128), dtype=np.float32)
    for h in range(2):
        for k in range(3):
            hk = 3 * h + k
            w_pf = conv_w[:, h::2, k].T  # [32 p, 128 f]
            wmat[:, 128 * hk:128 * (hk + 1)] = np.tile(w_pf, (4, 1))
    wmat = wmat.astype(np_bf16)

    bias = conv_b.reshape(128, 1)

    in_maps = []
    for j in range(num_devices):
        ids_core = ids_flat[j * words:(j + 1) * words]  # [words, 32]
        ids_g = ids_core.reshape(NGROUPS, 16, 4, 2, C)  # [g, blk, jh, o, c]

        # SWDGE pair stream for blocks 0..BS-1:
        # element m (group-local) = 128*blk + 32*jh + c
        pid = ids_g[:, :BS, :, 0, :] * V + ids_g[:, :BS, :, 1, :]
        flat = pid.reshape(-1).astype(np.int16)  # [(g, blk, jh, c)]
        wrapped = flat.reshape(-1, 16).T.copy()  # [16, ncols]
        idx = np.tile(wrapped, (8, 1))  # replicate to 128 partitions

        # one-hot for PE blocks BS..15: col = blockg*128 + 32*jh + c with
        # blockg = g*PE_BLOCKS + (blk-BS)*2 + o
        ids_pe = ids_g[:, BS:, :, :, :]  # [g, 16-BS blk, jh, o, c]
        ids_pe = ids_pe.transpose(0, 1, 3, 2, 4)  # [g, blk, o, jh, c]
        ids_pe = ids_pe.reshape(-1)  # [(g, blk, o, jh, c)] = [blocks*128]
        oh = (np.arange(V, dtype=np.int32)[:, None] == ids_pe[None, :])
        oh = np.ascontiguousarray(oh.astype(np_bf16))  # [101, blocks*128]

        in_maps.append(
            {
                "idx": np.ascontiguousarray(idx),
                "tab": ptab,
                "tabs": tab_bf,
                "oh": oh,
                "wmat": wmat,
                "bias": bias,
            }
        )
    return in_maps


def _ensure_ntff_hook():
    """The agent image's antenv lacks axon_hooks; shim it and install the
    ctypes NTFF profiling hook so trace=True yields HW exec times."""
    import types

    if "antenv.axon_hooks" in sys.modules:
        return
    mod = types.ModuleType("antenv.axon_hooks")
    _hook = [None]
    mod.get_axon_ntff_profile_hook = lambda: _hook[0]
    mod.set_axon_ntff_profile_hook = lambda h: _hook.__setitem__(0, h)
    sys.modules["antenv.axon_hooks"] = mod
    try:
        import antenv

        antenv.axon_hooks = mod
        from trn_agent_boot.trn_boot import _ntff_profile_via_ctypes

        hook = _ntff_profile_via_ctypes("/opt/axon/libaxon_pjrt.so")
        mod.set_axon_ntff_profile_hook(hook)
    except Exception as e:  # degrade to no-trace
        print(f"ntff hook install failed: {e}", file=sys.stderr)


_NC_CACHE = {}


def _get_nc():
    if "nc" not in _NC_CACHE:
        _NC_CACHE["nc"] = build_kernel()
    return _NC_CACHE["nc"]


def unscramble_out(raw, words=WORDS):
    """[128 f, words], col = 64c + 16j + what, what = 2*blk + o
    -> word 64c + 8*blk + 2*j + o."""
    nchunks = words // CHUNK_WORDS
    o = raw.reshape(128, nchunks, 4, 8, 2)  # [f, c, j, blk, o]
    o = o.transpose(1, 3, 2, 4, 0)  # [c, blk, j, o, f]
    return np.ascontiguousarray(o.reshape(words, 128))


def kernel(char_ids, emb_table, conv_w, conv_b, trace=False):
    if trace:
        _ensure_ntff_hook()
    nc = _get_nc()
    in_maps = host_prep(char_ids, emb_table, conv_w, conv_b)
    res = run_bass_kernel_spmd(
        nc, in_maps, core_ids=list(range(NCORES)), trace=trace
    )
    outs = [unscramble_out(res.results[j]["out"]) for j in range(NCORES)]
    full = np.concatenate(outs, axis=0).reshape(B, S, F).astype(np.float32)
    if trace:
        return full, res
    return full
